# revision 1
# baseline (speedup 1.0000x reference)
"""Trainium2 Bass kernel for nn_Block_70952859730367 (dense transformer block).

Strategy (8 NeuronCores, SPMD, one launch):
  Phase A  (per core): QKV projections for this core's 2 heads (h=2c, 2c+1)
           over ALL B*T tokens, in [d, t] layout (fp32r matmuls, K=C=1024).
  Phase A' : PE-transpose V into [s, d] layout + ones column (for softmax denom).
  Phase B  : causal attention per (b, 512-token t-block): scoresT = K^T-chunks
           vs Q (row-packed 2 heads, concurrent), exp on ACT (no max-sub — scores
           are small), causal mask via DVE multiply with host-fed mask tiles
           (diagonal chunks processed first), attnV accumulation with an appended
           ones column giving the softmax denominator for free.
           Normalize via DVE reciprocal + gpsimd partition_broadcast.
  A2A      : AllToAll redistributes attention outputs: core c ends up with all
           1024 head-dims for ITS 512 tokens.
  Phase D  : proj + residual + SiLU MLP + residual, token-parallel (512 tokens
           per core), streaming Wproj/W1/W2 slabs from HBM.

All matmuls run as float32r (TF32-rate: 1 cyc/row at N>=512) with fp32 PSUM
accumulation. Residual stream kept in full fp32.
"""
import os
import numpy as np

import concourse.bass as bass
import concourse.tile as tile
from concourse import bacc, mybir
from concourse import bass_utils
from concourse.masks import make_identity

B, T, C = 2, 2048, 1024
H, HS, FF = 16, 64, 4096
NT = B * T                      # 4096 tokens, b-major
NCORES = 8
TOK = NT // NCORES              # 512 tokens per core
SCALE = HS ** -0.5              # 0.125

F32 = mybir.dt.float32
F32R = mybir.dt.float32r
AF = mybir.ActivationFunctionType
ALU = mybir.AluOpType

_PROGRAM = None
LAST_EXEC_NS = None


def _emit(nc, tc, io, use_collective=True, stop_after=None):
    xT, xTown, wqkv, wproj, w1t, w2t, b1, out_d = (
        io["xT"], io["xTown"], io["wqkv"], io["wproj"], io["w1t"], io["w2t"],
        io["b1"], io["out"])
    masks = io["masks"]
    from contextlib import ExitStack

    outer = ExitStack()
    const = outer.enter_context(tc.tile_pool(name="const", bufs=1))
    wqkv_sb = const.tile([128, 8, 384], F32R, tag="wqkv")

    def load_wqkv_d(d):
        nc.sync.dma_start(
            out=wqkv_sb[:, :, 128 * d:128 * (d + 1)],
            in_=wqkv.ap().rearrange("(k p) d -> p k d", p=128)
                [:, :, 128 * d:128 * (d + 1)].bitcast(F32R))

    load_wqkv_d(0)   # q weights only; k/v weight loads go after tb0's x chunks
    identity = const.tile([128, 128], F32, tag="ident")
    make_identity(nc, identity[:])
    b1_sb = const.tile([128, 32, 1], F32, tag="b1")
    xTown_sb = const.tile([128, 8, 512], F32, tag="xTown")
    masks_sb = const.tile([128, 4, 512], F32, tag="masks")

    # W streaming pool — open early so prefetch DMAs can run during attention.
    wpool = outer.enter_context(tc.tile_pool(name="wstream", bufs=8))

    # DRAM bounce for the collective
    dram = outer.enter_context(tc.tile_pool(name="dram", bufs=1, space="DRAM"))
    a2a_in = dram.tile([8, 128, 512], F32, tag="a2ai")
    a2a_out = dram.tile([8, 128, 512], F32, tag="a2ao")

    attn_scope = ExitStack()
    qkvpool = attn_scope.enter_context(tc.tile_pool(name="qkv", bufs=1))
    q_sb = [qkvpool.tile([128, 2048], F32R, tag=f"q{b}", name=f"q{b}") for b in range(2)]
    k_sb = [qkvpool.tile([128, 2048], F32R, tag=f"k{b}", name=f"k{b}") for b in range(2)]
    v_sb = [qkvpool.tile([128, 2048], F32, tag=f"v{b}", name=f"v{b}") for b in range(2)]
    vsd = [qkvpool.tile([128, 16, 130], F32R, tag=f"vsd{b}", name=f"vsd{b}") for b in range(2)]

    # ---------------- Phase A: QKV + V-transpose ----------------
    with ExitStack() as pa:
        xtp = pa.enter_context(tc.tile_pool(name="xt", bufs=20))
        qkvp = pa.enter_context(tc.tile_pool(name="qkvp", bufs=6, space="PSUM"))
        tp = pa.enter_context(tc.tile_pool(name="tp", bufs=2, space="PSUM"))

        def emit_vtrans(b):
            for sc in range(16):
                pt = tp.tile([128, 128], F32, tag="tp", name=f"pt{b}_{sc}")
                nc.tensor.transpose(pt[:], v_sb[b][:, 128 * sc:128 * (sc + 1)],
                                    identity[:])
                dstv = vsd[b][:, sc, :].rearrange("p (h q) -> p h q", h=2)[:, :, 0:64]
                srcv = pt[:].rearrange("p (h q) -> p h q", h=2)
                nc.vector.tensor_copy(dstv, srcv)
            vv = vsd[b][:].rearrange("p k (h q) -> p k h q", h=2)
            nc.gpsimd.affine_select(
                out=vv, in_=vv, compare_op=ALU.not_equal, fill=1.0,
                base=-64, channel_multiplier=0,
                pattern=[[0, 16], [0, 2], [1, 65]])

        for tb in range(8):              # b-major 512-token blocks
            b, j = tb // 4, tb % 4
            xts = []
            for k in range(8):
                xt_t = xtp.tile([128, 512], F32R, tag="xt", name=f"xt{tb}_{k}")
                nc.sync.dma_start(
                    out=xt_t,
                    in_=xT.ap()[128 * k:128 * (k + 1),
                                512 * tb:512 * (tb + 1)].bitcast(F32R))
                xts.append(xt_t[:])
            if tb == 0:
                load_wqkv_d(1)
                load_wqkv_d(2)
            for d in range(3):           # q, k, v
                ps = qkvp.tile([128, 512], F32, tag="qkvp")
                for k in range(8):
                    nc.tensor.matmul(ps[:],
                                     lhsT=wqkv_sb[:, k, 128 * d:128 * (d + 1)],
                                     rhs=xts[k],
                                     start=(k == 0), stop=(k == 7))
                dst = (q_sb, k_sb, v_sb)[d][b]
                nc.vector.tensor_copy(dst[:, 512 * j:512 * (j + 1)], ps[:])
            if j == 3:
                emit_vtrans(b)

    if stop_after == "a":
        attn_scope.close()
        outer.close()
        return
    # ---------------- Phase B: attention ----------------
    with ExitStack() as pb:
        scp = pb.enter_context(tc.tile_pool(name="scp", bufs=2, space="PSUM"))
        avp = pb.enter_context(tc.tile_pool(name="avp", bufs=4, space="PSUM"))
        ep = pb.enter_context(tc.tile_pool(name="ep", bufs=8))
        afp = pb.enter_context(tc.tile_pool(name="afp", bufs=4))
        rp = pb.enter_context(tc.tile_pool(name="rp", bufs=4))

        nc.sync.dma_start(out=masks_sb, in_=masks.ap())

        for b in range(2):
            for j in range(4):
                t0 = 512 * j
                kmax = 4 * (j + 1)
                av = [avp.tile([65, 512], F32, tag="av", name=f"av{b}_{j}_{_h}") for _h in range(2)]

                def emit_attnv(pend):
                    # emit strictly descending in k so the start=True matmul
                    # (k = kmax-1) is the first in the PSUM accumulation group
                    # and stop=True (k = 0) is the last.
                    e, h, k0, k1 = pend
                    for ki, kk in ((1, k1), (0, k0)):
                        nc.tensor.matmul(
                            av[h][:],
                            lhsT=vsd[b][:, kk, 65 * h:65 * (h + 1)],
                            rhs=e[:, ki, :],
                            start=(kk == kmax - 1), stop=(kk == 0),
                            skip_group_check=True)

                pending = []
                npairs = kmax // 2
                for pr in range(npairs - 1, -1, -1):   # diag pairs first
                    k0, k1 = 2 * pr, 2 * pr + 1
                    for h in range(2):
                        sp = scp.tile([128, 2, 512], F32, tag="sc")
                        for ki, k in enumerate((k0, k1)):
                            nc.tensor.matmul(
                                sp[:, ki, :],
                                lhsT=k_sb[b][64 * h:64 * (h + 1),
                                             128 * k:128 * (k + 1)],
                                rhs=q_sb[b][64 * h:64 * (h + 1), t0:t0 + 512],
                                start=True, stop=True, skip_group_check=True)
                        e = ep.tile([128, 2, 512], F32R, tag="e")
                        nc.scalar.activation(e[:], sp[:], AF.Exp, scale=SCALE)
                        for ki, k in enumerate((k0, k1)):
                            if 128 * (k + 1) > t0:   # diagonal chunk: mask
                                nc.vector.tensor_mul(e[:, ki, :], e[:, ki, :],
                                                     masks_sb[:, k - 4 * j, :])
                        pending.append((e, h, k0, k1))
                    while len(pending) > 2:
                        emit_attnv(pending.pop(0))
                for p in pending:
                    emit_attnv(p)

                blk = 4 * b + j
                for h in range(2):
                    r = rp.tile([1, 512], F32, tag="r")
                    nc.vector.reciprocal(r[:], av[h][64:65, :])
                    rb = rp.tile([64, 512], F32, tag="rb")
                    nc.gpsimd.partition_broadcast(rb[:], r[:])
                    af = afp.tile([64, 512], F32, tag="af")
                    nc.vector.tensor_mul(af[:], av[h][0:64, :], rb[:])
                    nc.sync.dma_start(out=a2a_in[blk, 64 * h:64 * (h + 1), :],
                                      in_=af[:])

    if stop_after == "b":
        attn_scope.close()
        outer.close()
        return
    attn_scope.close()

    # Prefetch first weight slabs BEFORE the collective so the SP DMA queue
    # isn't head-of-line blocked behind collective-dependent loads.
    wp_pre = []
    for cc in range(6):
        wp = wpool.tile([128, 8, 128], F32R, tag="w", name=f"wpre{cc}")
        nc.sync.dma_start(
            out=wp,
            in_=wproj.ap().rearrange("(k p) m -> p k m", p=128)
                [:, :, 128 * cc:128 * (cc + 1)].bitcast(F32R))
        wp_pre.append(wp)

    # ---------------- A2A ----------------
    if use_collective:
        nc.gpsimd.collective_compute(
            "AllToAll", ALU.bypass,
            replica_groups=[list(range(NCORES))],
            ins=[a2a_in.opt()], outs=[a2a_out.opt()])
    else:  # timing-estimation build: stand-in DMA with similar byte volume
        nc.sync.dma_start(out=a2a_out[:], in_=a2a_in[:])

    if stop_after == "c":
        outer.close()
        return
    # ---------------- Phase D: proj + residual + MLP ----------------
    with ExitStack() as pd:
        atnp = pd.enter_context(tc.tile_pool(name="atn", bufs=8))
        x2fp = pd.enter_context(tc.tile_pool(name="x2f", bufs=8))
        x2rp = pd.enter_context(tc.tile_pool(name="x2r", bufs=8))
        hp = pd.enter_context(tc.tile_pool(name="hp", bufs=32))
        outp = pd.enter_context(tc.tile_pool(name="outp", bufs=4))
        mmp = pd.enter_context(tc.tile_pool(name="mmp", bufs=3, space="PSUM"))

        nc.sync.dma_start(out=b1_sb,
                          in_=b1.ap().rearrange("(k p) o -> p k o", p=128))
        nc.sync.dma_start(out=xTown_sb,
                          in_=xTown.ap().rearrange("(k p) n -> p k n", p=128))
        atn = []
        for k in range(8):
            t = atnp.tile([128, 512], F32R, tag="atn", name=f"atn{k}")
            nc.gpsimd.dma_start(out=t, in_=a2a_out[k].bitcast(F32R))
            atn.append(t)

        # proj + residual
        x2f, x2r = [], []
        for cc in range(8):
            if cc < 6:
                wp = wp_pre[cc]
            else:
                wp = wpool.tile([128, 8, 128], F32R, tag="w", name=f"wp{cc}")
                nc.sync.dma_start(
                    out=wp,
                    in_=wproj.ap().rearrange("(k p) m -> p k m", p=128)
                        [:, :, 128 * cc:128 * (cc + 1)].bitcast(F32R))
            ps = mmp.tile([128, 512], F32, tag="mm")
            for k in range(8):
                nc.tensor.matmul(ps[:], lhsT=wp[:, k, :], rhs=atn[k][:],
                                 start=(k == 0), stop=(k == 7))
            xf = x2fp.tile([128, 512], F32, tag="x2f")
            nc.vector.tensor_add(xf[:], xTown_sb[:, cc, :], ps[:])
            xr = x2rp.tile([128, 512], F32R, tag="x2r")
            nc.vector.tensor_copy(xr[:], xf[:])
            x2f.append(xf)
            x2r.append(xr)

        # mm1 + silu
        hts = []
        for fc in range(32):
            wp = wpool.tile([128, 8, 128], F32R, tag="w")
            nc.sync.dma_start(
                out=wp,
                in_=w1t.ap().rearrange("(k p) m -> p k m", p=128)
                    [:, :, 128 * fc:128 * (fc + 1)].bitcast(F32R))
            ps = mmp.tile([128, 512], F32, tag="mm")
            for cc in range(8):
                nc.tensor.matmul(ps[:], lhsT=wp[:, cc, :], rhs=x2r[cc][:],
                                 start=(cc == 0), stop=(cc == 7))
            ht = hp.tile([128, 512], F32R, tag="h")
            nc.scalar.activation(ht[:], ps[:], AF.Silu, bias=b1_sb[:, fc, :])
            hts.append(ht)

        # mm2 + residual + output
        for cc in range(8):
            ps = mmp.tile([128, 512], F32, tag="mm")
            for quarter in range(4):
                w2p = wpool.tile([128, 8, 128], F32R, tag="w", name=f"w2q{cc}_{quarter}")
                nc.sync.dma_start(
                    out=w2p,
                    in_=w2t.ap().rearrange("(k p) m -> p k m", p=128)
                        [:, 8 * quarter:8 * (quarter + 1),
                         128 * cc:128 * (cc + 1)].bitcast(F32R))
                for f in range(8):
                    fc = 8 * quarter + f
                    nc.tensor.matmul(ps[:], lhsT=w2p[:, f, :], rhs=hts[fc][:],
                                     start=(fc == 0), stop=(fc == 31))
            ot = outp.tile([128, 512], F32, tag="out")
            nc.vector.tensor_add(ot[:], x2f[cc][:], ps[:])
            nc.sync.dma_start(out=out_d.ap()[128 * cc:128 * (cc + 1), :], in_=ot[:])

    outer.close()


def build(single_core=False, stop_after=None, repeats=1):
    global _PROGRAM
    if not single_core and repeats == 1 and _PROGRAM is not None:
        return _PROGRAM
    nc = bacc.Bacc("TRN2", target_bir_lowering=False, debug=False,
                   num_devices=1 if single_core else NCORES)
    io = {
        "xT": nc.dram_tensor("xT", [C, NT], F32, kind="ExternalInput"),
        "xTown": nc.dram_tensor("xTown", [C, TOK], F32, kind="ExternalInput"),
        "wqkv": nc.dram_tensor("wqkv", [C, 384], F32, kind="ExternalInput"),
        "wproj": nc.dram_tensor("wproj", [C, C], F32, kind="ExternalInput"),
        "w1t": nc.dram_tensor("w1t", [C, FF], F32, kind="ExternalInput"),
        "w2t": nc.dram_tensor("w2t", [FF, C], F32, kind="ExternalInput"),
        "b1": nc.dram_tensor("b1", [FF, 1], F32, kind="ExternalInput"),
        "masks": nc.dram_tensor("masks", [128, 4, 512], F32, kind="ExternalInput"),
        "out": nc.dram_tensor("out", [C, TOK], F32, kind="ExternalOutput"),
    }
    with tile.TileContext(nc) as tc:
        for _r in range(repeats):
            _emit(nc, tc, io, use_collective=not single_core,
                  stop_after=stop_after)
    nc.compile()
    if single_core or repeats != 1:
        return nc
    _PROGRAM = nc
    return nc


def kernel(x, Wq, Wk, Wv, Wproj, W1, b1, W2):
    global LAST_EXEC_NS
    x = np.asarray(x, np.float32)
    xT = np.ascontiguousarray(x.reshape(NT, C).T)
    wprojT = np.ascontiguousarray(np.asarray(Wproj, np.float32).T)
    w1t = np.ascontiguousarray(np.asarray(W1, np.float32).T)
    w2t = np.ascontiguousarray(np.asarray(W2, np.float32).T)
    b1v = np.ascontiguousarray(np.asarray(b1, np.float32).reshape(FF, 1))
    Wq = np.asarray(Wq, np.float32)
    Wk = np.asarray(Wk, np.float32)
    Wv = np.asarray(Wv, np.float32)

    s_i = np.arange(128)[:, None, None]
    kr_i = np.arange(4)[None, :, None]
    t_i = np.arange(512)[None, None, :]
    masks = (128 * kr_i + s_i <= t_i).astype(np.float32)

    in_maps = []
    for c in range(NCORES):
        h0, h1 = 2 * c, 2 * c + 1
        wqkv = np.ascontiguousarray(np.concatenate(
            [Wq[h0], Wq[h1], Wk[h0], Wk[h1], Wv[h0], Wv[h1]], axis=1))
        in_maps.append({
            "xT": xT,
            "xTown": np.ascontiguousarray(xT[:, TOK * c:TOK * (c + 1)]),
            "wqkv": wqkv,
            "wproj": wprojT, "w1t": w1t, "w2t": w2t, "b1": b1v,
            "masks": masks,
        })

    nc = build()
    res = bass_utils.run_bass_kernel_spmd(
        nc, in_maps, core_ids=list(range(NCORES)))

    full = np.empty((NT, C), np.float32)
    for c in range(NCORES):
        full[TOK * c:TOK * (c + 1), :] = res.results[c]["out"].T
    return full.reshape(B, T, C)



# revision 29
# speedup vs baseline: 1.2492x; 1.2492x over previous
"""Trainium2 Bass kernel for nn_Block_70952859730367 (dense transformer block).

Strategy (8 NeuronCores, SPMD, one launch):
  Phase A  (per core): q/k projections for this core's 2 heads (h=2c, 2c+1)
           over ALL B*T tokens in [d, t] layout via fp8 DoubleRow matmuls
           (weights host-split hi+lo at one power-of-2 scale, x single fp8);
           V^T computed DIRECTLY as [token, d] via DR matmuls with x chunks
           as lhsT (no PE transpose), written to vsd in fp8 with an fp8 ones
           column for the softmax denominator.
  Phase B  : causal attention per (b, 512-token block): scoresT in fp32r
           (q,k kept F32R), exp on ACT straight to fp8 (scores are small, no
           max-sub; weight scales folded into the exp scale), causal mask via
           width-trimmed fp8 multiplies alternating DVE/Pool, attnV as fp8
           DoubleRow over key-chunk PAIRS with the ones column giving the
           denominator. Normalize via DVE reciprocal + gpsimd broadcast.
  A2A      : AllToAll in fp8 (4x fewer bytes): core c ends up with all 1024
           head-dims for ITS 512 tokens.
  Phase D  : proj + residual + SiLU MLP + residual, token-parallel, all
           matmuls fp8 DoubleRow (proj/mm2 2-term W-split, mm1 3-term with
           device-side x2 hi/lo split), scale-corrections folded into
           scalar_tensor_tensor residual adds and the SiLU activation scale.

All fp8 is e4m3 with power-of-2 per-tensor weight scaling (weights sit in
subnormal range otherwise); hi+lo splits share one scale so both accumulate
in the same PSUM group. Residual stream kept in full fp32.
"""
import numpy as np
import ml_dtypes

import concourse.bass as bass
import concourse.tile as tile
from concourse import bacc, mybir
from concourse import bass_utils

B, T, C = 2, 2048, 1024
H, HS, FF = 16, 64, 4096
NT = B * T                      # 4096 tokens, b-major
NCORES = 8
TOK = NT // NCORES              # 512 tokens per core
SCALE = HS ** -0.5              # 0.125

F32 = mybir.dt.float32
F32R = mybir.dt.float32r
FP8 = mybir.dt.float8e4
AF = mybir.ActivationFunctionType
ALU = mybir.AluOpType
DR = mybir.MatmulPerfMode.DoubleRow
E4M3 = ml_dtypes.float8_e4m3

# power-of-2 weight scales (host absmax is data-dependent but identical for
# every core; baked as compile-time immediates — computed in kernel() and
# passed into build()).
_PROGRAM = None
_PROG_SCALES = None
LAST_EXEC_NS = None


def _emit(nc, tc, io, scales, use_collective=True, stop_after=None):
    x8, wqk, wv8, wp8, w18, w28, b1_d, xTown, masks, out_d = (
        io["x8"], io["wqk"], io["wv8"], io["wp8"], io["w18"], io["w28"],
        io["b1"], io["xTown"], io["masks"], io["out"])
    s_q, s_k, s_v, s_p, s_1, s_2 = scales
    exp_scale = float(SCALE / (s_q * s_k))
    from contextlib import ExitStack

    outer = ExitStack()
    const = outer.enter_context(tc.tile_pool(name="const", bufs=1))
    wqk_sb = const.tile([128, 2, 4, 2, 2, 128], FP8, tag="wqk")
    wv_sb = const.tile([128, 2, 4, 2, 128], FP8, tag="wv")
    nc.sync.dma_start(out=wqk_sb, in_=wqk.ap())
    nc.sync.dma_start(out=wv_sb, in_=wv8.ap())
    masks_sb = const.tile([128, 4, 512], FP8, tag="masks")
    b1_sb = const.tile([128, 32, 1], F32, tag="b1")
    xtpool = outer.enter_context(tc.tile_pool(name="xt", bufs=6))
    xts = []
    for tb in range(8):
        xt = xtpool.tile([128, 2, 8, 512], FP8, tag="xt", name=f"xt{tb}")
        nc.sync.dma_start(out=xt, in_=x8.ap()[:, tb])
        xts.append(xt)
        if tb == 1:
            nc.sync.dma_start(out=masks_sb, in_=masks.ap())

    # phase-D weight streams on the Act HWDGE queue (issued early, consumed
    # late; separate queue avoids head-of-line blocking the phase A/B loads).
    w1pool = outer.enter_context(tc.tile_pool(name="w1s", bufs=6))
    w2pool = outer.enter_context(tc.tile_pool(name="w2s", bufs=2))
    wp_sb = const.tile([128, 8, 2, 4, 2, 128], FP8, tag="wp")
    w1_sl = [w1pool.tile([128, 4, 2, 4, 2, 128], FP8, tag="w1",
                         name=f"w1g{g}") for g in range(8)]
    nc.sync.dma_start(out=wp_sb, in_=wp8.ap())
    nc.sync.dma_start(out=b1_sb, in_=b1_d.ap())
    for g in range(8):
        nc.sync.dma_start(out=w1_sl[g], in_=w18.ap()[:, 4 * g:4 * (g + 1)])

    # DRAM bounce for the collective (fp8)
    dram = outer.enter_context(tc.tile_pool(name="dram", bufs=1, space="DRAM"))
    a2a_in = dram.tile([8, 128, 512], FP8, tag="a2ai")
    a2a_out = dram.tile([8, 128, 512], FP8, tag="a2ao")

    attn_scope = ExitStack()
    qkvpool = attn_scope.enter_context(tc.tile_pool(name="qkv", bufs=1))
    q_sb = [qkvpool.tile([128, 2048], F32R, tag=f"q{b}", name=f"q{b}")
            for b in range(2)]
    k_sb = [qkvpool.tile([128, 2048], F32R, tag=f"k{b}", name=f"k{b}")
            for b in range(2)]
    vsd = [qkvpool.tile([128, 16, 2, 128], FP8, tag=f"vsd{b}",
                        name=f"vsd{b}") for b in range(2)]
    for b in range(2):
        # cols 64..127 static per b: col 64 = 1.0 (softmax denominator via
        # the attnV matmul), cols 65.. = 0 so av rows 65..127 stay finite
        nc.gpsimd.memset(vsd[b][:, :, :, 64:128], 0.0)
        nc.gpsimd.memset(vsd[b][:, :, :, 64:65], 1.0)

    # ---------------- Phases A+B, interleaved emission ----------------
    # A(b=0) first, then B(b=0, j) interleaved with A(b=1) blocks so the
    # ACT-bound attention of b=0 overlaps the PE-bound projections of b=1.
    ab = ExitStack()
    qkp = ab.enter_context(tc.tile_pool(name="qkp", bufs=1, space="PSUM"))
    scp = ab.enter_context(tc.tile_pool(name="scp", bufs=2, space="PSUM"))
    avp = ab.enter_context(tc.tile_pool(name="avp", bufs=3, space="PSUM"))
    ep = ab.enter_context(tc.tile_pool(name="ep", bufs=10))
    afp = ab.enter_context(tc.tile_pool(name="afp", bufs=2))
    rp = ab.enter_context(tc.tile_pool(name="rp", bufs=1))
    mask_tog = [0]

    def emit_a(tb):
        b, j = tb // 4, tb % 4
        xt = xts[tb]
        terms = ((0, 0), (1, 0), (0, 1))   # (x part, w part)
        for d in range(2):           # q, k
            ps = qkp.tile([128, 512], F32, tag="qkp",
                          name=f"qk{tb}_{d}")[:]
            nmm = 0
            for xl, hl in terms:
                for p in range(4):
                    nmm += 1
                    nc.tensor.matmul(
                        ps[:],
                        lhsT=wqk_sb[:, hl, p, d, :, :],
                        rhs=xt[:, xl, 2 * p:2 * p + 2, :],
                        start=(nmm == 1), stop=(nmm == 12),
                        perf_mode=DR)
            dst = (q_sb, k_sb)[d][b]
            nc.vector.tensor_copy(dst[:, 512 * j:512 * (j + 1)], ps)
        for i in range(4):           # V^T per 128-token chunk
            vt = qkp.tile([128, 512], F32, tag="qkp",
                          name=f"vt{tb}_{i}")[:]
            nmm = 0
            for xl, hl in terms:
                for p in range(4):
                    nmm += 1
                    nc.tensor.matmul(
                        vt[:, 0:128],
                        lhsT=xt[:, xl, 2 * p:2 * p + 2,
                                128 * i:128 * (i + 1)],
                        rhs=wv_sb[:, hl, p, :, :],
                        start=(nmm == 1), stop=(nmm == 12),
                        perf_mode=DR)
            sc = 4 * j + i
            dstv = vsd[b][:, sc, :, 0:64]
            srcv = vt[:, 0:128].rearrange("p (h q) -> p h q", h=2)
            nc.vector.tensor_scalar_mul(dstv, srcv, float(1.0 / s_v))

    pending = []

    def emit_attnv(pend):
        e, h, pr, b, j, av = pend
        npairs = 2 * (j + 1)
        nc.tensor.matmul(
            av[h][:],
            lhsT=vsd[b][:, 2 * pr:2 * pr + 2, h, :],
            rhs=e[:],
            start=(pr == npairs - 1), stop=(pr == 0),
            perf_mode=DR, skip_group_check=True)

    def emit_b_scores(b, j):
        t0 = 512 * j
        kmax = 4 * (j + 1)
        npairs = kmax // 2
        av = [avp.tile([128, 512], F32, tag="av",
                       name=f"av{b}_{j}_{_h}") for _h in range(2)]
        for pr in range(npairs - 1, -1, -1):   # diag pairs first
            k0, k1 = 2 * pr, 2 * pr + 1
            m0, m1 = k0 - 4 * j, k1 - 4 * j
            for h in range(2):
                sp = scp.tile([128, 2, 512], F32, tag="sc",
                              name=f"sp{b}_{j}_{pr}_{h}")
                for ki, k in enumerate((k0, k1)):
                    nc.tensor.matmul(
                        sp[:, ki, :],
                        lhsT=k_sb[b][64 * h:64 * (h + 1),
                                     128 * k:128 * (k + 1)],
                        rhs=q_sb[b][64 * h:64 * (h + 1), t0:t0 + 512],
                        start=True, stop=True, skip_group_check=True)
                e = ep.tile([128, 2, 512], FP8, tag="e")
                if m0 >= 2:
                    # top diagonal pair: exp only the causal-reachable
                    # columns, zero the rest, mask the 128-wide triangle
                    for ki, m in ((0, m0), (1, m1)):
                        nc.scalar.activation(
                            e[:, ki, 128 * m:512], sp[:, ki, 128 * m:512],
                            AF.Exp, scale=exp_scale)
                        nc.gpsimd.memset(e[:, ki, 0:128 * m], 0.0)
                else:
                    nc.scalar.activation(e[:], sp[:], AF.Exp,
                                         scale=exp_scale)
                for ki, m in ((0, m0), (1, m1)):
                    if m >= 0:   # diagonal-block chunk: mask
                        lo = 128 * m if m >= 2 else 0
                        hi = 128 * (m + 1)
                        eng = (nc.vector, nc.gpsimd)[mask_tog[0] % 2]
                        mask_tog[0] += 1
                        eng.tensor_mul(e[:, ki, lo:hi], e[:, ki, lo:hi],
                                       masks_sb[:, m, lo:hi])
                pending.append((e, h, pr, b, j, av))
            while len(pending) > 4:
                emit_attnv(pending.pop(0))
        return av

    def emit_b_tail(b, j, av):
        while pending and pending[0][4] == j and pending[0][3] == b:
            emit_attnv(pending.pop(0))
        blk = 4 * b + j
        for h in range(2):
            r = rp.tile([1, 512], F32, tag="r")
            nc.vector.reciprocal(r[:], av[h][64:65, :])
            rb = rp.tile([64, 512], F32, tag="rb")
            nc.gpsimd.partition_broadcast(rb[:], r[:])
            af = afp.tile([64, 512], FP8, tag="af")
            nc.vector.tensor_mul(af[:], av[h][0:64, :], rb[:])
            nc.sync.dma_start(
                out=a2a_in[blk, 64 * h:64 * (h + 1), :], in_=af[:])

    if stop_after == "a":
        for tb in range(8):
            emit_a(tb)
        ab.close()
        attn_scope.close()
        outer.close()
        return
    # pipelined schedule: next block's scores are emitted before the
    # previous block's trailing attnVs so the ACT exp stream never starves
    emit_a(0)
    emit_a(1)
    av_prev = emit_b_scores(0, 0)
    prev = (0, 0)
    seq = [("a", 2), ("b", (0, 1)), ("a", 3), ("b", (0, 2)),
           ("b", (0, 3)), ("a", 4), ("b", (1, 0)), ("a", 5),
           ("b", (1, 1)), ("a", 6), ("b", (1, 2)), ("a", 7),
           ("b", (1, 3))]
    for kind, arg in seq:
        if kind == "a":
            emit_a(arg)
        else:
            b, j = arg
            av_new = emit_b_scores(b, j)
            emit_b_tail(*prev, av_prev)
            av_prev, prev = av_new, (b, j)
    emit_b_tail(*prev, av_prev)
    ab.close()

    if stop_after == "b":
        attn_scope.close()
        outer.close()
        return
    attn_scope.close()

    # ---------------- A2A ----------------
    if use_collective:
        nc.gpsimd.collective_compute(
            "AllToAll", ALU.bypass,
            replica_groups=[list(range(NCORES))],
            ins=[a2a_in.opt()], outs=[a2a_out.opt()])
    else:  # timing-estimation build: stand-in DMA with the same byte volume
        nc.sync.dma_start(
            out=a2a_out[:].rearrange("s p t -> p s t"),
            in_=a2a_in[:].rearrange("s p t -> p s t"))

    xtownp = outer.enter_context(tc.tile_pool(name="xtp2", bufs=1))
    xTown_sb = xtownp.tile([128, 8, 512], F32, tag="xTown")
    nc.scalar.dma_start(out=xTown_sb, in_=xTown.ap())
    # w2 slabs: issue now (Act queue) so transfers overlap mm1 compute
    w2_sl = []
    for cc in range(8):
        w2t_ = w2pool.tile([128, 2, 16, 2, 128], FP8, tag="w2",
                           name=f"w2c{cc}")
        nc.scalar.dma_start(out=w2t_, in_=w28.ap()[:, cc])
        w2_sl.append(w2t_)

    if stop_after == "c":
        outer.close()
        return
    # ---------------- Phase D: proj + residual + MLP ----------------
    with ExitStack() as pd:
        atnp = pd.enter_context(tc.tile_pool(name="atn", bufs=1))
        x2p = pd.enter_context(tc.tile_pool(name="x2", bufs=1))
        hp = pd.enter_context(tc.tile_pool(name="hp", bufs=1))  # h8 16KB
        outp = pd.enter_context(tc.tile_pool(name="outp", bufs=2))
        mmp = pd.enter_context(tc.tile_pool(name="mmp", bufs=3, space="PSUM"))

        atn = atnp.tile([128, 8, 512], FP8, tag="atn")
        nc.sync.dma_start(out=atn,
                          in_=a2a_out[:].rearrange("s p t -> p s t"))

        x2f = x2p.tile([128, 8, 512], F32, tag="x2f")
        x2q = x2p.tile([128, 8, 512], FP8, tag="x2q")
        x2lo = x2p.tile([128, 8, 512], FP8, tag="x2lo")
        h8 = hp.tile([128, 32, 512], FP8, tag="h8")

        # proj + residual + x2 quantize/split
        for cc in range(8):
            ps = mmp.tile([128, 512], F32, tag="mm")
            nmm = 0
            for hl in range(2):
                for p in range(4):
                    nmm += 1
                    nc.tensor.matmul(
                        ps[:], lhsT=wp_sb[:, cc, hl, p, :, :],
                        rhs=atn[:, 2 * p:2 * p + 2, :],
                        start=(nmm == 1), stop=(nmm == 8), perf_mode=DR)
            nc.vector.scalar_tensor_tensor(
                out=x2f[:, cc, :], in0=ps[:], scalar=float(1.0 / s_p),
                in1=xTown_sb[:, cc, :], op0=ALU.mult, op1=ALU.add)
            nc.gpsimd.tensor_copy(x2q[:, cc, :], x2f[:, cc, :])
            nc.vector.scalar_tensor_tensor(
                out=x2lo[:, cc, :], in0=x2q[:, cc, :], scalar=-1.0,
                in1=x2f[:, cc, :], op0=ALU.mult, op1=ALU.add)

        # mm1 (3-term) + silu -> fp8 h
        for fc in range(32):
            wt = w1_sl[fc // 4]
            fi = fc % 4
            ps = mmp.tile([128, 512], F32, tag="mm")
            nmm = 0
            for rhs_t, hl in ((x2q, 0), (x2lo, 0), (x2q, 1)):
                for p in range(4):
                    nmm += 1
                    nc.tensor.matmul(
                        ps[:], lhsT=wt[:, fi, hl, p, :, :],
                        rhs=rhs_t[:, 2 * p:2 * p + 2, :],
                        start=(nmm == 1), stop=(nmm == 12), perf_mode=DR)
            nc.scalar.activation(h8[:, fc, :], ps[:], AF.Silu,
                                 scale=float(1.0 / s_1),
                                 bias=b1_sb[:, fc, :])

        # mm2 (2-term W-split) + residual + output
        for cc in range(8):
            w2t_ = w2_sl[cc]
            ps = mmp.tile([128, 512], F32, tag="mm")
            nmm = 0
            for hl in range(2):
                for pf in range(16):
                    nmm += 1
                    nc.tensor.matmul(
                        ps[:], lhsT=w2t_[:, hl, pf, :, :],
                        rhs=h8[:, 2 * pf:2 * pf + 2, :],
                        start=(nmm == 1), stop=(nmm == 32), perf_mode=DR)
            ot = outp.tile([128, 512], F32, tag="out")
            nc.vector.scalar_tensor_tensor(
                out=ot[:], in0=ps[:], scalar=float(1.0 / s_2),
                in1=x2f[:, cc, :], op0=ALU.mult, op1=ALU.add)
            nc.sync.dma_start(out=out_d.ap()[128 * cc:128 * (cc + 1), :],
                              in_=ot[:])

    outer.close()


def build(scales=None, single_core=False, stop_after=None, repeats=1):
    global _PROGRAM, _PROG_SCALES
    if scales is None:
        scales = _PROG_SCALES or (2048.0,) * 6
    if (not single_core and repeats == 1 and _PROGRAM is not None
            and _PROG_SCALES == tuple(scales)):
        return _PROGRAM
    nc = bacc.Bacc("TRN2", target_bir_lowering=False, debug=False,
                   num_devices=1 if single_core else NCORES)
    io = {
        "x8": nc.dram_tensor("x8", [128, 8, 2, 8, 512], FP8,
                             kind="ExternalInput"),
        "wqk": nc.dram_tensor("wqk", [128, 2, 4, 2, 2, 128], FP8,
                              kind="ExternalInput"),
        "wv8": nc.dram_tensor("wv8", [128, 2, 4, 2, 128], FP8,
                              kind="ExternalInput"),
        "wp8": nc.dram_tensor("wp8", [128, 8, 2, 4, 2, 128], FP8,
                              kind="ExternalInput"),
        "w18": nc.dram_tensor("w18", [128, 32, 2, 4, 2, 128], FP8,
                              kind="ExternalInput"),
        "w28": nc.dram_tensor("w28", [128, 8, 2, 16, 2, 128], FP8,
                              kind="ExternalInput"),
        "b1": nc.dram_tensor("b1", [128, 32, 1], F32, kind="ExternalInput"),
        "xTown": nc.dram_tensor("xTown", [128, 8, 512], F32,
                                kind="ExternalInput"),
        "masks": nc.dram_tensor("masks", [128, 4, 512], FP8,
                                kind="ExternalInput"),
        "out": nc.dram_tensor("out", [C, TOK], F32, kind="ExternalOutput"),
    }
    with tile.TileContext(nc) as tc:
        for _r in range(repeats):
            _emit(nc, tc, io, scales, use_collective=not single_core,
                  stop_after=stop_after)
    nc.compile()
    if single_core or repeats != 1:
        return nc
    _PROGRAM = nc
    _PROG_SCALES = tuple(scales)
    return nc


def _pow2_scale(w, target=192.0):
    m = float(np.abs(w).max())
    return float(2.0 ** np.floor(np.log2(target / max(m, 1e-30))))


def _q8(a):
    return np.asarray(a, np.float32).astype(E4M3)


def _hilo(w, s):
    """hi/lo fp8 split of w*s, both parts at the same scale s."""
    ws = np.asarray(w, np.float32) * s
    hi = _q8(ws)
    lo = _q8(ws - hi.astype(np.float32))
    return hi, lo


def _pack_lhst(w, s, ncol_grp, pairs):
    """w [K, M] -> [128, ncol_grp, 2(hl), pairs, 2, 128] fp8 (lhsT DR layout).

    out[p_, g, hl, p, m, col] = (hi|lo)[(2p+m)*128 + p_, g*128 + col]
    """
    K, M = w.shape
    assert K == pairs * 256 and M == ncol_grp * 128
    hi, lo = _hilo(w, s)

    def lay(a):
        # [K, M] -> [pairs, 2, 128, ncol_grp, 128] -> [128, ncol_grp, pairs, 2, 128]
        a = a.reshape(pairs, 2, 128, ncol_grp, 128)
        return a.transpose(2, 3, 0, 1, 4)
    out = np.stack([lay(hi), lay(lo)], axis=2)  # [128, g, hl, pairs, 2, 128]
    return np.ascontiguousarray(out.transpose(0, 1, 2, 3, 4, 5))


def kernel(x, Wq, Wk, Wv, Wproj, W1, b1, W2):
    global LAST_EXEC_NS
    x = np.asarray(x, np.float32)
    xT = np.ascontiguousarray(x.reshape(NT, C).T)          # [C, NT]
    Wq = np.asarray(Wq, np.float32)
    Wk = np.asarray(Wk, np.float32)
    Wv = np.asarray(Wv, np.float32)
    WprojT = np.asarray(Wproj, np.float32).T               # [d 1024, c 1024]
    W1t = np.asarray(W1, np.float32).T                     # [C, FF]
    W2t = np.asarray(W2, np.float32).T                     # [FF, C]
    b1v = np.asarray(b1, np.float32).reshape(FF, 1)

    s_q = _pow2_scale(Wq)
    s_k = _pow2_scale(Wk)
    s_v = _pow2_scale(Wv)
    s_p = _pow2_scale(WprojT)
    s_1 = _pow2_scale(W1t)
    s_2 = _pow2_scale(W2t)
    scales = (s_q, s_k, s_v, s_p, s_1, s_2)

    # x8: [128, tb, hl, kc, 512] hi/lo fp8 split (same scale)
    xhi = _q8(xT)
    xlo = _q8(xT - xhi.astype(np.float32))
    x8 = np.stack([a.reshape(8, 128, 8, 512).transpose(1, 2, 0, 3)
                   for a in (xhi, xlo)], axis=2)
    x8 = np.ascontiguousarray(x8)
    # shared weights
    wp8 = _pack_lhst(WprojT, s_p, 8, 4)
    w18 = _pack_lhst(W1t, s_1, 32, 4)
    w28 = _pack_lhst(W2t, s_2, 8, 16)
    b1_h = np.ascontiguousarray(
        b1v.reshape(32, 128, 1).transpose(1, 0, 2))

    s_i = np.arange(128)[:, None, None]
    m_i = np.arange(4)[None, :, None]
    t_i = np.arange(512)[None, None, :]
    masks = _q8((128 * m_i + s_i <= t_i).astype(np.float32))

    in_maps = []
    for c in range(NCORES):
        h0, h1 = 2 * c, 2 * c + 1
        qcols = np.concatenate([Wq[h0], Wq[h1]], axis=1)   # [C, 128]
        kcols = np.concatenate([Wk[h0], Wk[h1]], axis=1)
        vcols = np.concatenate([Wv[h0], Wv[h1]], axis=1)
        # wqk: [128, hl, p, d, m, col]
        pq = _pack_lhst(qcols, s_q, 1, 4)  # [128, 1, 2, 4, 2, 128]
        pk = _pack_lhst(kcols, s_k, 1, 4)
        wqk_c = np.stack([pq[:, 0], pk[:, 0]], axis=3)  # [128,hl,p,d,2,128]
        pv = _pack_lhst(vcols, s_v, 1, 4)[:, 0]         # [128, hl, p, 2, 128]
        in_maps.append({
            "x8": x8,
            "wqk": np.ascontiguousarray(wqk_c),
            "wv8": np.ascontiguousarray(pv),
            "wp8": wp8, "w18": w18, "w28": w28, "b1": b1_h,
            "xTown": np.ascontiguousarray(
                xT[:, TOK * c:TOK * (c + 1)].reshape(8, 128, 512)
                .transpose(1, 0, 2)),
            "masks": masks,
        })

    nc = build(scales)
    res = bass_utils.run_bass_kernel_spmd(
        nc, in_maps, core_ids=list(range(NCORES)))

    full = np.empty((NT, C), np.float32)
    for c in range(NCORES):
        full[TOK * c:TOK * (c + 1), :] = res.results[c]["out"].T
    return full.reshape(B, T, C)


# revision 38
# speedup vs baseline: 1.3369x; 1.0702x over previous
"""Trainium2 Bass kernel for nn_Block_70952859730367 (dense transformer block).

Strategy (8 NeuronCores, SPMD, one launch):
  Phase A  (per core): q/k projections for this core's 2 heads (h=2c, 2c+1)
           over ALL B*T tokens in [d, t] layout via fp8 DoubleRow matmuls
           (weights host-split hi+lo at one power-of-2 scale, x single fp8);
           V^T computed DIRECTLY as [token, d] via DR matmuls with x chunks
           as lhsT (no PE transpose), written to vsd in fp8 with an fp8 ones
           column for the softmax denominator.
  Phase B  : causal attention per (b, 512-token block): scoresT in fp32r
           (q,k kept F32R), exp on ACT straight to fp8 (scores are small, no
           max-sub; weight scales folded into the exp scale), causal mask via
           width-trimmed fp8 multiplies alternating DVE/Pool, attnV as fp8
           DoubleRow over key-chunk PAIRS with the ones column giving the
           denominator. Normalize via DVE reciprocal + gpsimd broadcast.
  A2A      : AllToAll in fp8 (4x fewer bytes): core c ends up with all 1024
           head-dims for ITS 512 tokens.
  Phase D  : proj + residual + SiLU MLP + residual, token-parallel, all
           matmuls fp8 DoubleRow (proj/mm2 2-term W-split, mm1 3-term with
           device-side x2 hi/lo split), scale-corrections folded into
           scalar_tensor_tensor residual adds and the SiLU activation scale.

All fp8 is e4m3 with power-of-2 per-tensor weight scaling (weights sit in
subnormal range otherwise); hi+lo splits share one scale so both accumulate
in the same PSUM group. Residual stream kept in full fp32.
"""
import numpy as np
import ml_dtypes

import concourse.bass as bass
import concourse.tile as tile
from concourse import bacc, mybir
from concourse import bass_utils

B, T, C = 2, 2048, 1024
H, HS, FF = 16, 64, 4096
NT = B * T                      # 4096 tokens, b-major
NCORES = 8
TOK = NT // NCORES              # 512 tokens per core
SCALE = HS ** -0.5              # 0.125

F32 = mybir.dt.float32
F32R = mybir.dt.float32r
FP8 = mybir.dt.float8e4
AF = mybir.ActivationFunctionType
ALU = mybir.AluOpType
DR = mybir.MatmulPerfMode.DoubleRow
E4M3 = ml_dtypes.float8_e4m3

# power-of-2 weight scales (host absmax is data-dependent but identical for
# every core; baked as compile-time immediates — computed in kernel() and
# passed into build()).
_PROGRAM = None
_PROG_SCALES = None
LAST_EXEC_NS = None


def _emit(nc, tc, io, scales, use_collective=True, stop_after=None):
    x8, wqk, wv8, wp8, w18, w28, b1_d, xTown, masks, out_d = (
        io["x8"], io["wqk"], io["wv8"], io["wp8"], io["w18"], io["w28"],
        io["b1"], io["xTown"], io["masks"], io["out"])
    s_q, s_k, s_v, s_p, s_1, s_2 = scales
    exp_scale = float(SCALE / (s_q * s_k))
    from contextlib import ExitStack

    outer = ExitStack()
    const = outer.enter_context(tc.tile_pool(name="const", bufs=1))
    wqk_sb = const.tile([128, 2, 4, 2, 2, 128], FP8, tag="wqk")
    wv_sb = const.tile([128, 2, 4, 2, 128], FP8, tag="wv")
    nc.sync.dma_start(out=wqk_sb, in_=wqk.ap())
    nc.sync.dma_start(out=wv_sb, in_=wv8.ap())
    masks_sb = const.tile([128, 4, 512], FP8, tag="masks")
    b1_sb = const.tile([128, 32, 1], F32, tag="b1")
    xtpool = outer.enter_context(tc.tile_pool(name="xt", bufs=6))
    xts = []
    for tb in range(8):
        xt = xtpool.tile([128, 2, 8, 512], FP8, tag="xt", name=f"xt{tb}")
        nc.sync.dma_start(out=xt, in_=x8.ap()[:, tb])
        xts.append(xt)
        if tb == 1:
            nc.sync.dma_start(out=masks_sb, in_=masks.ap())

    # phase-D weight streams on the Act HWDGE queue (issued early, consumed
    # late; separate queue avoids head-of-line blocking the phase A/B loads).
    w1pool = outer.enter_context(tc.tile_pool(name="w1s", bufs=6))
    w2pool = outer.enter_context(tc.tile_pool(name="w2s", bufs=6))
    wp_sb = const.tile([128, 8, 1, 4, 2, 128], FP8, tag="wp")
    w1_sl = [w1pool.tile([128, 4, 2, 4, 2, 128], FP8, tag="w1",
                         name=f"w1g{g}") for g in range(8)]
    nc.sync.dma_start(out=wp_sb, in_=wp8.ap()[:, :, 0:1])
    nc.sync.dma_start(out=b1_sb, in_=b1_d.ap())
    for g in range(8):
        nc.sync.dma_start(out=w1_sl[g], in_=w18.ap()[:, 4 * g:4 * (g + 1)])

    # DRAM bounce for the collective (fp8)
    dram = outer.enter_context(tc.tile_pool(name="dram", bufs=1, space="DRAM"))
    a2a_in = dram.tile([8, 128, 512], FP8, tag="a2ai")
    a2a_out = dram.tile([8, 128, 512], FP8, tag="a2ao")

    attn_scope = ExitStack()
    qkvpool = attn_scope.enter_context(tc.tile_pool(name="qkv", bufs=1))
    q_sb = [qkvpool.tile([128, 2048], F32R, tag=f"q{b}", name=f"q{b}")
            for b in range(2)]
    k_sb = [qkvpool.tile([128, 2048], F32R, tag=f"k{b}", name=f"k{b}")
            for b in range(2)]
    vsd = [qkvpool.tile([128, 16, 2, 128], FP8, tag=f"vsd{b}",
                        name=f"vsd{b}") for b in range(2)]
    for b in range(2):
        # cols 64..127 static per b: col 64 = 1.0 (softmax denominator via
        # the attnV matmul), cols 65.. = 0 so av rows 65..127 stay finite
        nc.gpsimd.memset(vsd[b][:, :, :, 64:128], 0.0)
        nc.gpsimd.memset(vsd[b][:, :, :, 64:65], 1.0)

    # ---------------- Phases A+B, interleaved emission ----------------
    # A(b=0) first, then B(b=0, j) interleaved with A(b=1) blocks so the
    # ACT-bound attention of b=0 overlaps the PE-bound projections of b=1.
    ab = ExitStack()
    qkp = ab.enter_context(tc.tile_pool(name="qkp", bufs=1, space="PSUM"))
    scp = ab.enter_context(tc.tile_pool(name="scp", bufs=2, space="PSUM"))
    avp = ab.enter_context(tc.tile_pool(name="avp", bufs=3, space="PSUM"))
    ep = ab.enter_context(tc.tile_pool(name="ep", bufs=10))
    afp = ab.enter_context(tc.tile_pool(name="afp", bufs=2))
    rp = ab.enter_context(tc.tile_pool(name="rp", bufs=1))
    mask_tog = [0]

    def emit_a(tb):
        b, j = tb // 4, tb % 4
        xt = xts[tb]
        terms = ((0, 0), (1, 0), (0, 1))   # (x part, w part)
        for d in range(2):           # q, k
            ps = qkp.tile([128, 512], F32, tag="qkp",
                          name=f"qk{tb}_{d}")[:]
            nmm = 0
            for xl, hl in terms:
                for p in range(4):
                    nmm += 1
                    nc.tensor.matmul(
                        ps[:],
                        lhsT=wqk_sb[:, hl, p, d, :, :],
                        rhs=xt[:, xl, 2 * p:2 * p + 2, :],
                        start=(nmm == 1), stop=(nmm == 12),
                        perf_mode=DR)
            dst = (q_sb, k_sb)[d][b]
            nc.vector.tensor_copy(dst[:, 512 * j:512 * (j + 1)], ps)
        for i in range(4):           # V^T per 128-token chunk
            vt = qkp.tile([128, 512], F32, tag="qkp",
                          name=f"vt{tb}_{i}")[:]
            nmm = 0
            for xl, hl in terms:
                for p in range(4):
                    nmm += 1
                    nc.tensor.matmul(
                        vt[:, 0:128],
                        lhsT=xt[:, xl, 2 * p:2 * p + 2,
                                128 * i:128 * (i + 1)],
                        rhs=wv_sb[:, hl, p, :, :],
                        start=(nmm == 1), stop=(nmm == 12),
                        perf_mode=DR)
            sc = 4 * j + i
            dstv = vsd[b][:, sc, :, 0:64]
            srcv = vt[:, 0:128].rearrange("p (h q) -> p h q", h=2)
            nc.vector.tensor_scalar_mul(dstv, srcv, float(1.0 / s_v))

    pending = []

    def emit_attnv(pend):
        e, h, pr, b, j, av = pend
        npairs = 2 * (j + 1)
        nc.tensor.matmul(
            av[h][:],
            lhsT=vsd[b][:, 2 * pr:2 * pr + 2, h, :],
            rhs=e[:],
            start=(pr == npairs - 1), stop=(pr == 0),
            perf_mode=DR, skip_group_check=True)

    def emit_b_scores(b, j):
        t0 = 512 * j
        kmax = 4 * (j + 1)
        npairs = kmax // 2
        av = [avp.tile([128, 512], F32, tag="av",
                       name=f"av{b}_{j}_{_h}") for _h in range(2)]
        for pr in range(npairs - 1, -1, -1):   # diag pairs first
            k0, k1 = 2 * pr, 2 * pr + 1
            m0, m1 = k0 - 4 * j, k1 - 4 * j
            for h in range(2):
                sp = scp.tile([128, 2, 512], F32, tag="sc",
                              name=f"sp{b}_{j}_{pr}_{h}")
                for ki, k in enumerate((k0, k1)):
                    nc.tensor.matmul(
                        sp[:, ki, :],
                        lhsT=k_sb[b][64 * h:64 * (h + 1),
                                     128 * k:128 * (k + 1)],
                        rhs=q_sb[b][64 * h:64 * (h + 1), t0:t0 + 512],
                        start=True, stop=True, skip_group_check=True)
                e = ep.tile([128, 2, 512], FP8, tag="e")
                if m0 >= 2:
                    # top diagonal pair: exp only the causal-reachable
                    # columns, zero the rest, mask the 128-wide triangle
                    for ki, m in ((0, m0), (1, m1)):
                        nc.scalar.activation(
                            e[:, ki, 128 * m:512], sp[:, ki, 128 * m:512],
                            AF.Exp, scale=exp_scale)
                        nc.gpsimd.memset(e[:, ki, 0:128 * m], 0.0)
                else:
                    nc.scalar.activation(e[:], sp[:], AF.Exp,
                                         scale=exp_scale)
                for ki, m in ((0, m0), (1, m1)):
                    if m >= 0:   # diagonal-block chunk: mask
                        lo = 128 * m if m >= 2 else 0
                        hi = 128 * (m + 1)
                        eng = (nc.vector, nc.gpsimd)[mask_tog[0] % 2]
                        mask_tog[0] += 1
                        eng.tensor_mul(e[:, ki, lo:hi], e[:, ki, lo:hi],
                                       masks_sb[:, m, lo:hi])
                pending.append((e, h, pr, b, j, av))
            while len(pending) > 4:
                emit_attnv(pending.pop(0))
        return av

    def emit_b_tail(b, j, av):
        while pending and pending[0][4] == j and pending[0][3] == b:
            emit_attnv(pending.pop(0))
        blk = 4 * b + j
        for h in range(2):
            r = rp.tile([1, 512], F32, tag="r")
            nc.vector.reciprocal(r[:], av[h][64:65, :])
            rb = rp.tile([64, 512], F32, tag="rb")
            nc.gpsimd.partition_broadcast(rb[:], r[:])
            af = afp.tile([64, 512], FP8, tag="af")
            nc.vector.tensor_mul(af[:], av[h][0:64, :], rb[:])
            nc.sync.dma_start(
                out=a2a_in[blk, 64 * h:64 * (h + 1), :], in_=af[:])

    if stop_after == "a":
        for tb in range(8):
            emit_a(tb)
        ab.close()
        attn_scope.close()
        outer.close()
        return
    # pipelined schedule: next block's scores are emitted before the
    # previous block's trailing attnVs so the ACT exp stream never starves
    emit_a(0)
    emit_a(1)
    av_prev = emit_b_scores(0, 0)
    prev = (0, 0)
    seq = [("a", 2), ("b", (0, 1)), ("a", 3), ("b", (0, 2)),
           ("b", (0, 3)), ("a", 4), ("b", (1, 0)), ("a", 5),
           ("b", (1, 1)), ("a", 6), ("b", (1, 2)), ("a", 7),
           ("b", (1, 3))]
    for kind, arg in seq:
        if kind == "a":
            emit_a(arg)
        else:
            b, j = arg
            av_new = emit_b_scores(b, j)
            emit_b_tail(*prev, av_prev)
            av_prev, prev = av_new, (b, j)
    emit_b_tail(*prev, av_prev)
    ab.close()

    if stop_after == "b":
        attn_scope.close()
        outer.close()
        return
    attn_scope.close()

    # ---------------- A2A ----------------
    if use_collective:
        nc.gpsimd.collective_compute(
            "AllToAll", ALU.bypass,
            replica_groups=[list(range(NCORES))],
            ins=[a2a_in.opt()], outs=[a2a_out.opt()])
    else:  # timing-estimation build: stand-in DMA with the same byte volume
        nc.sync.dma_start(
            out=a2a_out[:].rearrange("s p t -> p s t"),
            in_=a2a_in[:].rearrange("s p t -> p s t"))

    xtownp = outer.enter_context(tc.tile_pool(name="xtp2", bufs=1))
    xTown_sb = xtownp.tile([128, 8, 512], F32, tag="xTown")
    nc.scalar.dma_start(out=xTown_sb, in_=xTown.ap())
    # w2 half-slabs (per cc, per hl): finer stream pacing during mm1/mm2
    w2_sl = []
    for cc in range(8):
        halves = []
        for hl in range(2):
            w2t_ = w2pool.tile([128, 16, 2, 128], FP8, tag="w2",
                               name=f"w2c{cc}_{hl}")
            nc.scalar.dma_start(out=w2t_, in_=w28.ap()[:, cc, hl])
            halves.append(w2t_)
        w2_sl.append(halves)

    if stop_after == "c":
        outer.close()
        return
    # ---------------- Phase D: proj + residual + MLP ----------------
    with ExitStack() as pd:
        atnp = pd.enter_context(tc.tile_pool(name="atn", bufs=1))
        x2p = pd.enter_context(tc.tile_pool(name="x2", bufs=1))
        hp = pd.enter_context(tc.tile_pool(name="hp", bufs=1))  # h8 16KB
        outp = pd.enter_context(tc.tile_pool(name="outp", bufs=2))
        mmp = pd.enter_context(tc.tile_pool(name="mmp", bufs=3, space="PSUM"))

        atn = atnp.tile([128, 8, 512], FP8, tag="atn")
        nc.sync.dma_start(out=atn,
                          in_=a2a_out[:].rearrange("s p t -> p s t"))

        x2f = x2p.tile([128, 8, 512], F32, tag="x2f")
        x2q = x2p.tile([128, 8, 512], FP8, tag="x2q")
        x2lo = x2p.tile([128, 8, 512], FP8, tag="x2lo")
        h8 = hp.tile([128, 32, 512], FP8, tag="h8")

        # proj + residual + x2 quantize/split (W hi-term only: attn is
        # already the dominant quantization error on this path)
        for cc in range(8):
            ps = mmp.tile([128, 512], F32, tag="mm")
            for p in range(4):
                nc.tensor.matmul(
                    ps[:], lhsT=wp_sb[:, cc, 0, p, :, :],
                    rhs=atn[:, 2 * p:2 * p + 2, :],
                    start=(p == 0), stop=(p == 3), perf_mode=DR)
            nc.vector.scalar_tensor_tensor(
                out=x2f[:, cc, :], in0=ps[:], scalar=float(1.0 / s_p),
                in1=xTown_sb[:, cc, :], op0=ALU.mult, op1=ALU.add)
            nc.gpsimd.tensor_copy(x2q[:, cc, :], x2f[:, cc, :])
            nc.vector.scalar_tensor_tensor(
                out=x2lo[:, cc, :], in0=x2q[:, cc, :], scalar=-1.0,
                in1=x2f[:, cc, :], op0=ALU.mult, op1=ALU.add)

        # mm1 (3-term) + silu -> fp8 h
        for fc in range(32):
            wt = w1_sl[fc // 4]
            fi = fc % 4
            ps = mmp.tile([128, 512], F32, tag="mm")
            nmm = 0
            for rhs_t, hl in ((x2q, 0), (x2lo, 0), (x2q, 1)):
                for p in range(4):
                    nmm += 1
                    nc.tensor.matmul(
                        ps[:], lhsT=wt[:, fi, hl, p, :, :],
                        rhs=rhs_t[:, 2 * p:2 * p + 2, :],
                        start=(nmm == 1), stop=(nmm == 12), perf_mode=DR)
            nc.scalar.activation(h8[:, fc, :], ps[:], AF.Silu,
                                 scale=float(1.0 / s_1),
                                 bias=b1_sb[:, fc, :])

        # mm2 (2-term W-split) + residual + output
        for cc in range(8):
            ps = mmp.tile([128, 512], F32, tag="mm")
            nmm = 0
            for hl in range(2):
                for pf in range(16):
                    nmm += 1
                    nc.tensor.matmul(
                        ps[:], lhsT=w2_sl[cc][hl][:, pf, :, :],
                        rhs=h8[:, 2 * pf:2 * pf + 2, :],
                        start=(nmm == 1), stop=(nmm == 32), perf_mode=DR)
            ot = outp.tile([128, 512], F32, tag="out")
            nc.vector.scalar_tensor_tensor(
                out=ot[:], in0=ps[:], scalar=float(1.0 / s_2),
                in1=x2f[:, cc, :], op0=ALU.mult, op1=ALU.add)
            nc.sync.dma_start(out=out_d.ap()[128 * cc:128 * (cc + 1), :],
                              in_=ot[:])

    outer.close()


def build(scales=None, single_core=False, stop_after=None, repeats=1):
    global _PROGRAM, _PROG_SCALES
    if scales is None:
        scales = _PROG_SCALES or (2048.0,) * 6
    if (not single_core and repeats == 1 and _PROGRAM is not None
            and _PROG_SCALES == tuple(scales)):
        return _PROGRAM
    nc = bacc.Bacc("TRN2", target_bir_lowering=False, debug=False,
                   num_devices=1 if single_core else NCORES)
    io = {
        "x8": nc.dram_tensor("x8", [128, 8, 2, 8, 512], FP8,
                             kind="ExternalInput"),
        "wqk": nc.dram_tensor("wqk", [128, 2, 4, 2, 2, 128], FP8,
                              kind="ExternalInput"),
        "wv8": nc.dram_tensor("wv8", [128, 2, 4, 2, 128], FP8,
                              kind="ExternalInput"),
        "wp8": nc.dram_tensor("wp8", [128, 8, 2, 4, 2, 128], FP8,
                              kind="ExternalInput"),
        "w18": nc.dram_tensor("w18", [128, 32, 2, 4, 2, 128], FP8,
                              kind="ExternalInput"),
        "w28": nc.dram_tensor("w28", [128, 8, 2, 16, 2, 128], FP8,
                              kind="ExternalInput"),
        "b1": nc.dram_tensor("b1", [128, 32, 1], F32, kind="ExternalInput"),
        "xTown": nc.dram_tensor("xTown", [128, 8, 512], F32,
                                kind="ExternalInput"),
        "masks": nc.dram_tensor("masks", [128, 4, 512], FP8,
                                kind="ExternalInput"),
        "out": nc.dram_tensor("out", [C, TOK], F32, kind="ExternalOutput"),
    }
    with tile.TileContext(nc) as tc:
        for _r in range(repeats):
            _emit(nc, tc, io, scales, use_collective=not single_core,
                  stop_after=stop_after)
    nc.compile()
    if single_core or repeats != 1:
        return nc
    _PROGRAM = nc
    _PROG_SCALES = tuple(scales)
    return nc


def _pow2_scale(w, target=192.0):
    m = float(np.abs(w).max())
    return float(2.0 ** np.floor(np.log2(target / max(m, 1e-30))))


def _q8(a):
    return np.asarray(a, np.float32).astype(E4M3)


def _hilo(w, s):
    """hi/lo fp8 split of w*s, both parts at the same scale s."""
    ws = np.asarray(w, np.float32) * s
    hi = _q8(ws)
    lo = _q8(ws - hi.astype(np.float32))
    return hi, lo


def _pack_lhst(w, s, ncol_grp, pairs):
    """w [K, M] -> [128, ncol_grp, 2(hl), pairs, 2, 128] fp8 (lhsT DR layout).

    out[p_, g, hl, p, m, col] = (hi|lo)[(2p+m)*128 + p_, g*128 + col]
    """
    K, M = w.shape
    assert K == pairs * 256 and M == ncol_grp * 128
    hi, lo = _hilo(w, s)

    def lay(a):
        # [K, M] -> [pairs, 2, 128, ncol_grp, 128] -> [128, ncol_grp, pairs, 2, 128]
        a = a.reshape(pairs, 2, 128, ncol_grp, 128)
        return a.transpose(2, 3, 0, 1, 4)
    out = np.stack([lay(hi), lay(lo)], axis=2)  # [128, g, hl, pairs, 2, 128]
    return np.ascontiguousarray(out.transpose(0, 1, 2, 3, 4, 5))


def kernel(x, Wq, Wk, Wv, Wproj, W1, b1, W2):
    global LAST_EXEC_NS
    x = np.asarray(x, np.float32)
    xT = np.ascontiguousarray(x.reshape(NT, C).T)          # [C, NT]
    Wq = np.asarray(Wq, np.float32)
    Wk = np.asarray(Wk, np.float32)
    Wv = np.asarray(Wv, np.float32)
    WprojT = np.asarray(Wproj, np.float32).T               # [d 1024, c 1024]
    W1t = np.asarray(W1, np.float32).T                     # [C, FF]
    W2t = np.asarray(W2, np.float32).T                     # [FF, C]
    b1v = np.asarray(b1, np.float32).reshape(FF, 1)

    s_q = _pow2_scale(Wq)
    s_k = _pow2_scale(Wk)
    s_v = _pow2_scale(Wv)
    s_p = _pow2_scale(WprojT)
    s_1 = _pow2_scale(W1t)
    s_2 = _pow2_scale(W2t)
    scales = (s_q, s_k, s_v, s_p, s_1, s_2)

    # x8: [128, tb, hl, kc, 512] hi/lo fp8 split (same scale)
    xhi = _q8(xT)
    xlo = _q8(xT - xhi.astype(np.float32))
    x8 = np.stack([a.reshape(8, 128, 8, 512).transpose(1, 2, 0, 3)
                   for a in (xhi, xlo)], axis=2)
    x8 = np.ascontiguousarray(x8)
    # shared weights
    wp8 = _pack_lhst(WprojT, s_p, 8, 4)
    w18 = _pack_lhst(W1t, s_1, 32, 4)
    w28 = _pack_lhst(W2t, s_2, 8, 16)
    b1_h = np.ascontiguousarray(
        b1v.reshape(32, 128, 1).transpose(1, 0, 2))

    s_i = np.arange(128)[:, None, None]
    m_i = np.arange(4)[None, :, None]
    t_i = np.arange(512)[None, None, :]
    masks = _q8((128 * m_i + s_i <= t_i).astype(np.float32))

    in_maps = []
    for c in range(NCORES):
        h0, h1 = 2 * c, 2 * c + 1
        qcols = np.concatenate([Wq[h0], Wq[h1]], axis=1)   # [C, 128]
        kcols = np.concatenate([Wk[h0], Wk[h1]], axis=1)
        vcols = np.concatenate([Wv[h0], Wv[h1]], axis=1)
        # wqk: [128, hl, p, d, m, col]
        pq = _pack_lhst(qcols, s_q, 1, 4)  # [128, 1, 2, 4, 2, 128]
        pk = _pack_lhst(kcols, s_k, 1, 4)
        wqk_c = np.stack([pq[:, 0], pk[:, 0]], axis=3)  # [128,hl,p,d,2,128]
        pv = _pack_lhst(vcols, s_v, 1, 4)[:, 0]         # [128, hl, p, 2, 128]
        in_maps.append({
            "x8": x8,
            "wqk": np.ascontiguousarray(wqk_c),
            "wv8": np.ascontiguousarray(pv),
            "wp8": wp8, "w18": w18, "w28": w28, "b1": b1_h,
            "xTown": np.ascontiguousarray(
                xT[:, TOK * c:TOK * (c + 1)].reshape(8, 128, 512)
                .transpose(1, 0, 2)),
            "masks": masks,
        })

    nc = build(scales)
    res = bass_utils.run_bass_kernel_spmd(
        nc, in_maps, core_ids=list(range(NCORES)))

    full = np.empty((NT, C), np.float32)
    for c in range(NCORES):
        full[TOK * c:TOK * (c + 1), :] = res.results[c]["out"].T
    return full.reshape(B, T, C)


# revision 40
# speedup vs baseline: 1.3710x; 1.0255x over previous
"""Trainium2 Bass kernel for nn_Block_70952859730367 (dense transformer block).

Strategy (8 NeuronCores, SPMD, one launch):
  Phase A  (per core): q/k projections for this core's 2 heads (h=2c, 2c+1)
           over ALL B*T tokens in [d, t] layout via fp8 DoubleRow matmuls
           (weights host-split hi+lo at one power-of-2 scale, x single fp8);
           V^T computed DIRECTLY as [token, d] via DR matmuls with x chunks
           as lhsT (no PE transpose), written to vsd in fp8 with an fp8 ones
           column for the softmax denominator.
  Phase B  : causal attention per (b, 512-token block): scoresT in fp32r
           (q,k kept F32R), exp on ACT straight to fp8 (scores are small, no
           max-sub; weight scales folded into the exp scale), causal mask via
           width-trimmed fp8 multiplies alternating DVE/Pool, attnV as fp8
           DoubleRow over key-chunk PAIRS with the ones column giving the
           denominator. Normalize via DVE reciprocal + gpsimd broadcast.
  A2A      : AllToAll in fp8 (4x fewer bytes): core c ends up with all 1024
           head-dims for ITS 512 tokens.
  Phase D  : proj + residual + SiLU MLP + residual, token-parallel, all
           matmuls fp8 DoubleRow (proj/mm2 2-term W-split, mm1 3-term with
           device-side x2 hi/lo split), scale-corrections folded into
           scalar_tensor_tensor residual adds and the SiLU activation scale.

All fp8 is e4m3 with power-of-2 per-tensor weight scaling (weights sit in
subnormal range otherwise); hi+lo splits share one scale so both accumulate
in the same PSUM group. Residual stream kept in full fp32.
"""
import numpy as np
import ml_dtypes

import concourse.bass as bass
import concourse.tile as tile
from concourse import bacc, mybir
from concourse import bass_utils

B, T, C = 2, 2048, 1024
H, HS, FF = 16, 64, 4096
NT = B * T                      # 4096 tokens, b-major
NCORES = 8
TOK = NT // NCORES              # 512 tokens per core
SCALE = HS ** -0.5              # 0.125

F32 = mybir.dt.float32
F32R = mybir.dt.float32r
FP8 = mybir.dt.float8e4
AF = mybir.ActivationFunctionType
ALU = mybir.AluOpType
DR = mybir.MatmulPerfMode.DoubleRow
E4M3 = ml_dtypes.float8_e4m3

# power-of-2 weight scales (host absmax is data-dependent but identical for
# every core; baked as compile-time immediates — computed in kernel() and
# passed into build()).
_PROGRAM = None
_PROG_SCALES = None
LAST_EXEC_NS = None


def _emit(nc, tc, io, scales, use_collective=True, stop_after=None):
    x8, wqk, wv8, wp8, w18, w28, b1_d, xTown, masks, out_d = (
        io["x8"], io["wqk"], io["wv8"], io["wp8"], io["w18"], io["w28"],
        io["b1"], io["xTown"], io["masks"], io["out"])
    s_q, s_k, s_v, s_p, s_1, s_2 = scales
    exp_scale = float(SCALE / (s_q * s_k))
    from contextlib import ExitStack

    outer = ExitStack()
    const = outer.enter_context(tc.tile_pool(name="const", bufs=1))
    wqk_sb = const.tile([128, 2, 4, 2, 2, 128], FP8, tag="wqk")
    wv_sb = const.tile([128, 2, 4, 2, 128], FP8, tag="wv")
    nc.sync.dma_start(out=wqk_sb, in_=wqk.ap())
    nc.sync.dma_start(out=wv_sb, in_=wv8.ap())
    masks_sb = const.tile([128, 4, 512], FP8, tag="masks")
    b1_sb = const.tile([128, 32, 1], F32, tag="b1")
    xtpool = outer.enter_context(tc.tile_pool(name="xt", bufs=6))
    xts = []
    for tb in range(8):
        xt = xtpool.tile([128, 2, 8, 512], FP8, tag="xt", name=f"xt{tb}")
        nc.sync.dma_start(out=xt, in_=x8.ap()[:, tb])
        xts.append(xt)
        if tb == 1:
            nc.sync.dma_start(out=masks_sb, in_=masks.ap())

    # phase-D weight streams on the Act HWDGE queue (issued early, consumed
    # late; separate queue avoids head-of-line blocking the phase A/B loads).
    w1pool = outer.enter_context(tc.tile_pool(name="w1s", bufs=4))
    w2pool = outer.enter_context(tc.tile_pool(name="w2s", bufs=6))
    wp_sb = const.tile([128, 8, 1, 4, 2, 128], FP8, tag="wp")
    w1_sl = [w1pool.tile([128, 4, 2, 4, 2, 128], FP8, tag="w1",
                         name=f"w1g{g}") for g in range(8)]
    nc.sync.dma_start(out=wp_sb, in_=wp8.ap()[:, :, 0:1])
    nc.sync.dma_start(out=b1_sb, in_=b1_d.ap())
    for g in range(8):
        nc.sync.dma_start(out=w1_sl[g], in_=w18.ap()[:, 4 * g:4 * (g + 1)])
    xtownp = outer.enter_context(tc.tile_pool(name="xtp2", bufs=1))
    xTown_sb = xtownp.tile([128, 8, 512], F32, tag="xTown")
    nc.sync.dma_start(out=xTown_sb, in_=xTown.ap())

    # DRAM bounce for the collective (fp8)
    dram = outer.enter_context(tc.tile_pool(name="dram", bufs=1, space="DRAM"))
    a2a_in = dram.tile([8, 128, 512], FP8, tag="a2ai")
    a2a_out = dram.tile([8, 128, 512], FP8, tag="a2ao")

    attn_scope = ExitStack()
    qkvpool = attn_scope.enter_context(tc.tile_pool(name="qkv", bufs=1))
    q_sb = [qkvpool.tile([128, 2048], F32R, tag=f"q{b}", name=f"q{b}")
            for b in range(2)]
    k_sb = [qkvpool.tile([128, 2048], F32R, tag=f"k{b}", name=f"k{b}")
            for b in range(2)]
    vsd = [qkvpool.tile([128, 16, 2, 128], FP8, tag=f"vsd{b}",
                        name=f"vsd{b}") for b in range(2)]
    for b in range(2):
        # cols 64..127 static per b: col 64 = 1.0 (softmax denominator via
        # the attnV matmul), cols 65.. = 0 so av rows 65..127 stay finite
        nc.gpsimd.memset(vsd[b][:, :, :, 64:128], 0.0)
        nc.gpsimd.memset(vsd[b][:, :, :, 64:65], 1.0)

    # ---------------- Phases A+B, interleaved emission ----------------
    # A(b=0) first, then B(b=0, j) interleaved with A(b=1) blocks so the
    # ACT-bound attention of b=0 overlaps the PE-bound projections of b=1.
    ab = ExitStack()
    qkp = ab.enter_context(tc.tile_pool(name="qkp", bufs=1, space="PSUM"))
    scp = ab.enter_context(tc.tile_pool(name="scp", bufs=2, space="PSUM"))
    avp = ab.enter_context(tc.tile_pool(name="avp", bufs=3, space="PSUM"))
    ep = ab.enter_context(tc.tile_pool(name="ep", bufs=10))
    afp = ab.enter_context(tc.tile_pool(name="afp", bufs=2))
    rp = ab.enter_context(tc.tile_pool(name="rp", bufs=1))
    mask_tog = [0]

    def emit_a(tb):
        b, j = tb // 4, tb % 4
        xt = xts[tb]
        terms = ((0, 0), (1, 0), (0, 1))   # (x part, w part)
        for d in range(2):           # q, k
            ps = qkp.tile([128, 512], F32, tag="qkp",
                          name=f"qk{tb}_{d}")[:]
            nmm = 0
            for xl, hl in terms:
                for p in range(4):
                    nmm += 1
                    nc.tensor.matmul(
                        ps[:],
                        lhsT=wqk_sb[:, hl, p, d, :, :],
                        rhs=xt[:, xl, 2 * p:2 * p + 2, :],
                        start=(nmm == 1), stop=(nmm == 12),
                        perf_mode=DR)
            dst = (q_sb, k_sb)[d][b]
            nc.vector.tensor_copy(dst[:, 512 * j:512 * (j + 1)], ps)
        for i in range(4):           # V^T per 128-token chunk
            vt = qkp.tile([128, 512], F32, tag="qkp",
                          name=f"vt{tb}_{i}")[:]
            nmm = 0
            for xl, hl in terms:
                for p in range(4):
                    nmm += 1
                    nc.tensor.matmul(
                        vt[:, 0:128],
                        lhsT=xt[:, xl, 2 * p:2 * p + 2,
                                128 * i:128 * (i + 1)],
                        rhs=wv_sb[:, hl, p, :, :],
                        start=(nmm == 1), stop=(nmm == 12),
                        perf_mode=DR)
            sc = 4 * j + i
            dstv = vsd[b][:, sc, :, 0:64]
            srcv = vt[:, 0:128].rearrange("p (h q) -> p h q", h=2)
            nc.vector.tensor_scalar_mul(dstv, srcv, float(1.0 / s_v))

    pending = []

    def emit_attnv(pend):
        e, h, pr, b, j, av = pend
        npairs = 2 * (j + 1)
        nc.tensor.matmul(
            av[h][:],
            lhsT=vsd[b][:, 2 * pr:2 * pr + 2, h, :],
            rhs=e[:],
            start=(pr == npairs - 1), stop=(pr == 0),
            perf_mode=DR, skip_group_check=True)

    def emit_b_scores(b, j):
        t0 = 512 * j
        kmax = 4 * (j + 1)
        npairs = kmax // 2
        av = [avp.tile([128, 512], F32, tag="av",
                       name=f"av{b}_{j}_{_h}") for _h in range(2)]
        for pr in range(npairs - 1, -1, -1):   # diag pairs first
            k0, k1 = 2 * pr, 2 * pr + 1
            m0, m1 = k0 - 4 * j, k1 - 4 * j
            for h in range(2):
                sp = scp.tile([128, 2, 512], F32, tag="sc",
                              name=f"sp{b}_{j}_{pr}_{h}")
                for ki, k in enumerate((k0, k1)):
                    nc.tensor.matmul(
                        sp[:, ki, :],
                        lhsT=k_sb[b][64 * h:64 * (h + 1),
                                     128 * k:128 * (k + 1)],
                        rhs=q_sb[b][64 * h:64 * (h + 1), t0:t0 + 512],
                        start=True, stop=True, skip_group_check=True)
                e = ep.tile([128, 2, 512], FP8, tag="e")
                if m0 >= 2:
                    # top diagonal pair: exp only the causal-reachable
                    # columns, zero the rest, mask the 128-wide triangle
                    for ki, m in ((0, m0), (1, m1)):
                        nc.scalar.activation(
                            e[:, ki, 128 * m:512], sp[:, ki, 128 * m:512],
                            AF.Exp, scale=exp_scale)
                        nc.gpsimd.memset(e[:, ki, 0:128 * m], 0.0)
                else:
                    nc.scalar.activation(e[:], sp[:], AF.Exp,
                                         scale=exp_scale)
                for ki, m in ((0, m0), (1, m1)):
                    if m >= 0:   # diagonal-block chunk: mask
                        lo = 128 * m if m >= 2 else 0
                        hi = 128 * (m + 1)
                        eng = (nc.vector, nc.gpsimd)[mask_tog[0] % 2]
                        mask_tog[0] += 1
                        eng.tensor_mul(e[:, ki, lo:hi], e[:, ki, lo:hi],
                                       masks_sb[:, m, lo:hi])
                pending.append((e, h, pr, b, j, av))
            while len(pending) > 4:
                emit_attnv(pending.pop(0))
        return av

    def emit_b_tail(b, j, av):
        while pending and pending[0][4] == j and pending[0][3] == b:
            emit_attnv(pending.pop(0))
        blk = 4 * b + j
        for h in range(2):
            r = rp.tile([1, 512], F32, tag="r")
            nc.vector.reciprocal(r[:], av[h][64:65, :])
            rb = rp.tile([64, 512], F32, tag="rb")
            nc.gpsimd.partition_broadcast(rb[:], r[:])
            af = afp.tile([64, 512], FP8, tag="af")
            nc.vector.tensor_mul(af[:], av[h][0:64, :], rb[:])
            nc.sync.dma_start(
                out=a2a_in[blk, 64 * h:64 * (h + 1), :], in_=af[:])

    if stop_after == "a":
        for tb in range(8):
            emit_a(tb)
        ab.close()
        attn_scope.close()
        outer.close()
        return
    # pipelined schedule: next block's scores are emitted before the
    # previous block's trailing attnVs so the ACT exp stream never starves
    emit_a(0)
    emit_a(1)
    av_prev = emit_b_scores(0, 0)
    prev = (0, 0)
    seq = [("a", 2), ("b", (0, 1)), ("a", 3), ("b", (0, 2)),
           ("b", (0, 3)), ("a", 4), ("b", (1, 0)), ("a", 5),
           ("b", (1, 1)), ("a", 6), ("b", (1, 2)), ("a", 7),
           ("b", (1, 3))]
    for kind, arg in seq:
        if kind == "a":
            emit_a(arg)
        else:
            b, j = arg
            av_new = emit_b_scores(b, j)
            emit_b_tail(*prev, av_prev)
            av_prev, prev = av_new, (b, j)
    emit_b_tail(*prev, av_prev)
    ab.close()

    if stop_after == "b":
        attn_scope.close()
        outer.close()
        return
    attn_scope.close()

    # ---------------- A2A ----------------
    atnp = outer.enter_context(tc.tile_pool(name="atn", bufs=1))
    atn = atnp.tile([128, 8, 512], FP8, tag="atn")
    if use_collective:
        nc.gpsimd.collective_compute(
            "AllToAll", ALU.bypass,
            replica_groups=[list(range(NCORES))],
            ins=[a2a_in.opt()], outs=[a2a_out.opt()])
        nc.sync.dma_start(out=atn,
                          in_=a2a_out[:].rearrange("s p t -> p s t"))
    else:  # timing-estimation build: stand-in DMA with the same byte volume
        nc.sync.dma_start(out=atn,
                          in_=a2a_in[:].rearrange("s p t -> p s t"))

    # w2 half-slabs (per cc, per hl): finer stream pacing during mm1/mm2
    w2_sl = []
    for cc in range(8):
        halves = []
        for hl in range(2):
            w2t_ = w2pool.tile([128, 16, 2, 128], FP8, tag="w2",
                               name=f"w2c{cc}_{hl}")
            nc.scalar.dma_start(out=w2t_, in_=w28.ap()[:, cc, hl])
            halves.append(w2t_)
        w2_sl.append(halves)

    if stop_after == "c":
        outer.close()
        return
    # ---------------- Phase D: proj + residual + MLP ----------------
    with ExitStack() as pd:
        x2p = pd.enter_context(tc.tile_pool(name="x2", bufs=1))
        hp = pd.enter_context(tc.tile_pool(name="hp", bufs=1))  # h8 16KB
        outp = pd.enter_context(tc.tile_pool(name="outp", bufs=2))
        mmp = pd.enter_context(tc.tile_pool(name="mmp", bufs=3, space="PSUM"))

        x2f = x2p.tile([128, 8, 512], F32, tag="x2f")
        x2q = x2p.tile([128, 8, 512], FP8, tag="x2q")
        x2lo = x2p.tile([128, 8, 512], FP8, tag="x2lo")
        h8 = hp.tile([128, 32, 512], FP8, tag="h8")

        # proj + residual + x2 quantize/split (W hi-term only: attn is
        # already the dominant quantization error on this path)
        for cc in range(8):
            ps = mmp.tile([128, 512], F32, tag="mm")
            for p in range(4):
                nc.tensor.matmul(
                    ps[:], lhsT=wp_sb[:, cc, 0, p, :, :],
                    rhs=atn[:, 2 * p:2 * p + 2, :],
                    start=(p == 0), stop=(p == 3), perf_mode=DR)
            nc.vector.scalar_tensor_tensor(
                out=x2f[:, cc, :], in0=ps[:], scalar=float(1.0 / s_p),
                in1=xTown_sb[:, cc, :], op0=ALU.mult, op1=ALU.add)
            nc.gpsimd.tensor_copy(x2q[:, cc, :], x2f[:, cc, :])
            nc.vector.scalar_tensor_tensor(
                out=x2lo[:, cc, :], in0=x2q[:, cc, :], scalar=-1.0,
                in1=x2f[:, cc, :], op0=ALU.mult, op1=ALU.add)

        # mm1 (3-term) + silu -> fp8 h
        for fc in range(32):
            wt = w1_sl[fc // 4]
            fi = fc % 4
            ps = mmp.tile([128, 512], F32, tag="mm")
            nmm = 0
            for rhs_t, hl in ((x2q, 0), (x2lo, 0), (x2q, 1)):
                for p in range(4):
                    nmm += 1
                    nc.tensor.matmul(
                        ps[:], lhsT=wt[:, fi, hl, p, :, :],
                        rhs=rhs_t[:, 2 * p:2 * p + 2, :],
                        start=(nmm == 1), stop=(nmm == 12), perf_mode=DR)
            nc.scalar.activation(h8[:, fc, :], ps[:], AF.Silu,
                                 scale=float(1.0 / s_1),
                                 bias=b1_sb[:, fc, :])

        # mm2 (2-term W-split) + residual + output
        for cc in range(8):
            ps = mmp.tile([128, 512], F32, tag="mm")
            nmm = 0
            for hl in range(2):
                for pf in range(16):
                    nmm += 1
                    nc.tensor.matmul(
                        ps[:], lhsT=w2_sl[cc][hl][:, pf, :, :],
                        rhs=h8[:, 2 * pf:2 * pf + 2, :],
                        start=(nmm == 1), stop=(nmm == 32), perf_mode=DR)
            ot = outp.tile([128, 512], F32, tag="out")
            nc.vector.scalar_tensor_tensor(
                out=ot[:], in0=ps[:], scalar=float(1.0 / s_2),
                in1=x2f[:, cc, :], op0=ALU.mult, op1=ALU.add)
            nc.sync.dma_start(out=out_d.ap()[128 * cc:128 * (cc + 1), :],
                              in_=ot[:])

    outer.close()


def build(scales=None, single_core=False, stop_after=None, repeats=1):
    global _PROGRAM, _PROG_SCALES
    if scales is None:
        scales = _PROG_SCALES or (2048.0,) * 6
    if (not single_core and repeats == 1 and _PROGRAM is not None
            and _PROG_SCALES == tuple(scales)):
        return _PROGRAM
    nc = bacc.Bacc("TRN2", target_bir_lowering=False, debug=False,
                   num_devices=1 if single_core else NCORES)
    io = {
        "x8": nc.dram_tensor("x8", [128, 8, 2, 8, 512], FP8,
                             kind="ExternalInput"),
        "wqk": nc.dram_tensor("wqk", [128, 2, 4, 2, 2, 128], FP8,
                              kind="ExternalInput"),
        "wv8": nc.dram_tensor("wv8", [128, 2, 4, 2, 128], FP8,
                              kind="ExternalInput"),
        "wp8": nc.dram_tensor("wp8", [128, 8, 2, 4, 2, 128], FP8,
                              kind="ExternalInput"),
        "w18": nc.dram_tensor("w18", [128, 32, 2, 4, 2, 128], FP8,
                              kind="ExternalInput"),
        "w28": nc.dram_tensor("w28", [128, 8, 2, 16, 2, 128], FP8,
                              kind="ExternalInput"),
        "b1": nc.dram_tensor("b1", [128, 32, 1], F32, kind="ExternalInput"),
        "xTown": nc.dram_tensor("xTown", [128, 8, 512], F32,
                                kind="ExternalInput"),
        "masks": nc.dram_tensor("masks", [128, 4, 512], FP8,
                                kind="ExternalInput"),
        "out": nc.dram_tensor("out", [C, TOK], F32, kind="ExternalOutput"),
    }
    with tile.TileContext(nc) as tc:
        for _r in range(repeats):
            _emit(nc, tc, io, scales, use_collective=not single_core,
                  stop_after=stop_after)
    nc.compile()
    if single_core or repeats != 1:
        return nc
    _PROGRAM = nc
    _PROG_SCALES = tuple(scales)
    return nc


def _pow2_scale(w, target=192.0):
    m = float(np.abs(w).max())
    return float(2.0 ** np.floor(np.log2(target / max(m, 1e-30))))


def _q8(a):
    return np.asarray(a, np.float32).astype(E4M3)


def _hilo(w, s):
    """hi/lo fp8 split of w*s, both parts at the same scale s."""
    ws = np.asarray(w, np.float32) * s
    hi = _q8(ws)
    lo = _q8(ws - hi.astype(np.float32))
    return hi, lo


def _pack_lhst(w, s, ncol_grp, pairs):
    """w [K, M] -> [128, ncol_grp, 2(hl), pairs, 2, 128] fp8 (lhsT DR layout).

    out[p_, g, hl, p, m, col] = (hi|lo)[(2p+m)*128 + p_, g*128 + col]
    """
    K, M = w.shape
    assert K == pairs * 256 and M == ncol_grp * 128
    hi, lo = _hilo(w, s)

    def lay(a):
        # [K, M] -> [pairs, 2, 128, ncol_grp, 128] -> [128, ncol_grp, pairs, 2, 128]
        a = a.reshape(pairs, 2, 128, ncol_grp, 128)
        return a.transpose(2, 3, 0, 1, 4)
    out = np.stack([lay(hi), lay(lo)], axis=2)  # [128, g, hl, pairs, 2, 128]
    return np.ascontiguousarray(out.transpose(0, 1, 2, 3, 4, 5))


def kernel(x, Wq, Wk, Wv, Wproj, W1, b1, W2):
    global LAST_EXEC_NS
    x = np.asarray(x, np.float32)
    xT = np.ascontiguousarray(x.reshape(NT, C).T)          # [C, NT]
    Wq = np.asarray(Wq, np.float32)
    Wk = np.asarray(Wk, np.float32)
    Wv = np.asarray(Wv, np.float32)
    WprojT = np.asarray(Wproj, np.float32).T               # [d 1024, c 1024]
    W1t = np.asarray(W1, np.float32).T                     # [C, FF]
    W2t = np.asarray(W2, np.float32).T                     # [FF, C]
    b1v = np.asarray(b1, np.float32).reshape(FF, 1)

    s_q = _pow2_scale(Wq)
    s_k = _pow2_scale(Wk)
    s_v = _pow2_scale(Wv)
    s_p = _pow2_scale(WprojT)
    s_1 = _pow2_scale(W1t)
    s_2 = _pow2_scale(W2t)
    scales = (s_q, s_k, s_v, s_p, s_1, s_2)

    # x8: [128, tb, hl, kc, 512] hi/lo fp8 split (same scale)
    xhi = _q8(xT)
    xlo = _q8(xT - xhi.astype(np.float32))
    x8 = np.stack([a.reshape(8, 128, 8, 512).transpose(1, 2, 0, 3)
                   for a in (xhi, xlo)], axis=2)
    x8 = np.ascontiguousarray(x8)
    # shared weights
    wp8 = _pack_lhst(WprojT, s_p, 8, 4)
    w18 = _pack_lhst(W1t, s_1, 32, 4)
    w28 = _pack_lhst(W2t, s_2, 8, 16)
    b1_h = np.ascontiguousarray(
        b1v.reshape(32, 128, 1).transpose(1, 0, 2))

    s_i = np.arange(128)[:, None, None]
    m_i = np.arange(4)[None, :, None]
    t_i = np.arange(512)[None, None, :]
    masks = _q8((128 * m_i + s_i <= t_i).astype(np.float32))

    in_maps = []
    for c in range(NCORES):
        h0, h1 = 2 * c, 2 * c + 1
        qcols = np.concatenate([Wq[h0], Wq[h1]], axis=1)   # [C, 128]
        kcols = np.concatenate([Wk[h0], Wk[h1]], axis=1)
        vcols = np.concatenate([Wv[h0], Wv[h1]], axis=1)
        # wqk: [128, hl, p, d, m, col]
        pq = _pack_lhst(qcols, s_q, 1, 4)  # [128, 1, 2, 4, 2, 128]
        pk = _pack_lhst(kcols, s_k, 1, 4)
        wqk_c = np.stack([pq[:, 0], pk[:, 0]], axis=3)  # [128,hl,p,d,2,128]
        pv = _pack_lhst(vcols, s_v, 1, 4)[:, 0]         # [128, hl, p, 2, 128]
        in_maps.append({
            "x8": x8,
            "wqk": np.ascontiguousarray(wqk_c),
            "wv8": np.ascontiguousarray(pv),
            "wp8": wp8, "w18": w18, "w28": w28, "b1": b1_h,
            "xTown": np.ascontiguousarray(
                xT[:, TOK * c:TOK * (c + 1)].reshape(8, 128, 512)
                .transpose(1, 0, 2)),
            "masks": masks,
        })

    nc = build(scales)
    res = bass_utils.run_bass_kernel_spmd(
        nc, in_maps, core_ids=list(range(NCORES)))

    full = np.empty((NT, C), np.float32)
    for c in range(NCORES):
        full[TOK * c:TOK * (c + 1), :] = res.results[c]["out"].T
    return full.reshape(B, T, C)
